# revision 13
# baseline (speedup 1.0000x reference)
"""Trainium2 Bass kernel for CrossFeature: out[b, p(i,j)] = x[b,i]*x[b,j]*dot(v[i],v[j]).

Full shapes: x [8192, 300] f32, v [300, 4] f32 -> out [8192, 44850] f32
(P = 300*299/2 upper-triangular pairs, row-major order).

Strategy (data-parallel over 8 NeuronCores, batch-sharded, no cross-core
communication; per core 1024 rows as [128 part, 8 bh, 300]):
  - host: w[p] = (v @ v.T)[i(p), j(p)] as fp16, reshaped [128, 351] so the
    whole table lives in SBUF from one tiny DMA (351 = 44928/128; P padded).
  - output columns in 351-aligned chunks. Per chunk:
      * PE broadcasts w into PSUM fp32: ones[1,128]^T @ w2[row, :351] per
        351-col block (w2 rows hold consecutive w slices; rhs partition
        offset = row).
      * pass 1 (t = x_i * x_seg): per segment, the 8 row-blocks (b) are
        split b-granularly across ScalarE (activation Copy w/ scale=x_i,
        per (i,b)), DVE and GPSIMD (tensor_tensor with stride-0 x_i
        broadcast over a contiguous b-range) by per-chunk time budgets, so
        every chunk has the same engine profile and all engines run below
        the DMA-bound chunk wall.
      * pass 2 (t *= w): one DVE TT [128, 8, cw_d] vs PSUM w (stride-0 mid
        dim) + GPSIMD TT on the last cw_gc columns from an SBUF w copy
        (ScalarE).
      * out DMA alternates between the two HWDGE rings (sync / scalar).
  - chunk schedule: small lead-in/lead-out pieces (fast pipeline ramp and
    drain), middle chunks interleaved (0,N-1,1,N-2,...) so ACT-heavy early
    chunks pair with GPSIMD-heavy late ones; pass-2/DMA of chunk k-1 and
    the PE broadcast of chunk k are emitted after pass-1 of chunk k
    (software pipelining; also makes the single PSUM buffer safe).
"""

import numpy as np

import concourse.bacc as bacc
import concourse.bass as bass
import concourse.mybir as mybir
from concourse.tile import TileContext
from concourse.bass_utils import run_bass_kernel_spmd

N_CORES = 8
B_FULL = 8192
F_FULL = 300

CHUNK = 2560          # output columns per main chunk
EDGE_PIECES = 640     # small pieces for pipeline lead-in/lead-out

# per-chunk engine budgets, scaled by cw/CHUNK
T_ACT_NS = 29000      # ScalarE pass-1 budget (517 ns/instr, count-bound)
T_GPS1_NS = 15000     # GPSIMD pass-1 budget (2.17 ns/elem)
ACT_UNIT_NS = 517
GPS_ELEM_NS = 2.17
P2G_FRAC = 0.285      # pass-2 column share for GPSIMD (vs DVE)


def bcast_last(ap, n):
    """[..., 1] AP -> [..., n] with stride-0 last dim (free-dim broadcast)."""
    a = [list(d) for d in ap.ap]
    assert a[-1][1] == 1, a
    return bass.AP(ap.tensor, ap.offset, a[:-1] + [[0, n]])


def bcast_mid(ap, n):
    """[p, m] AP -> [p, n, m] with a stride-0 middle dim."""
    a = [list(d) for d in ap.ap]
    return bass.AP(ap.tensor, ap.offset, a[:-1] + [[0, n]] + a[-1:])


def chunk_segments(f, c0, c1):
    """Pair-segments of the triu(f, k=1) row-major layout intersected with
    column window [c0, c1). Yields (i, ps, pe, j0): output cols [ps, pe) hold
    x[:, i] * x[:, j0 : j0 + (pe-ps)]."""
    s = 0
    for i in range(f - 1):
        ln = f - 1 - i
        s0, s1 = s, s + ln
        if s0 >= c1:
            break
        if s1 > c0:
            ps, pe = max(s0, c0), min(s1, c1)
            yield i, ps, pe, i + 1 + (ps - s0)
        s = s1


def chunk_schedule(p_pairs):
    """[(c0, c1), ...] in emission order: small lead-in pieces, interleaved
    middle chunks, small lead-out pieces. All boundaries 351-aligned."""
    bounds = []
    c = 0
    while c < p_pairs:
        first = c == 0
        step = EDGE_PIECES if first else CHUNK
        bounds.append((c, min(c + step, p_pairs)))
        c += step
    # split the last full chunk into small pieces too
    last = bounds.pop()
    c = last[0]
    while c < last[1]:
        bounds.append((c, min(c + EDGE_PIECES, last[1])))
        c += EDGE_PIECES
    n_lead = 1
    n_tail = len([b for b in bounds if b[0] >= last[0]])
    mid = bounds[n_lead:len(bounds) - n_tail]
    order = bounds[:n_lead]
    lo, hi = 0, len(mid) - 1
    while lo <= hi:
        order.append(mid[lo])
        if hi != lo:
            order.append(mid[hi])
        lo, hi = lo + 1, hi - 1
    order += bounds[len(bounds) - n_tail:]
    return order


def build_program(bh=8, f=F_FULL, n_cores=N_CORES):
    """Build + compile the per-core Bass program. Shard shape: [bh*128, f]."""
    p_pairs = f * (f - 1) // 2
    rows = bh * 128
    f32 = mybir.dt.float32
    f16 = mybir.dt.float16

    nc = bacc.Bacc("TRN2", target_bir_lowering=False, debug=False,
                   num_devices=n_cores)
    x_d = nc.dram_tensor("x", [rows, f], f32, kind="ExternalInput")
    w_d = nc.dram_tensor("w", [1, p_pairs], f16, kind="ExternalInput")
    o_d = nc.dram_tensor("out", [rows, p_pairs], f32, kind="ExternalOutput")

    sched = chunk_schedule(p_pairs)

    with TileContext(nc) as tc:
        with (
            tc.tile_pool(name="xp", bufs=1) as xp,
            tc.tile_pool(name="wp", bufs=3) as wp,
            tc.tile_pool(name="wb", bufs=2) as wb,
            tc.tile_pool(name="op", bufs=2) as op,
            tc.tile_pool(name="pp", bufs=1, space=bass.MemorySpace.PSUM) as pp,
        ):
            x_sb = xp.tile([128, bh, f], f32)
            nc.scalar.dma_start(
                out=x_sb[:],
                in_=x_d.rearrange("(bh bl) f -> bl bh f", bl=128),
            )
            ones = xp.tile([1, 128], f16)
            nc.vector.memset(ones[:], 1.0)

            out_r = o_d.rearrange("(bh bl) p -> bl bh p", bl=128)

            def emit_w_broadcast(st):
                _, c0, c1, cw, cw_d, cw_gc, w_ps, w_bc, w_sb = st
                for n0 in range(0, cw, 512):
                    n1 = min(n0 + 512, cw)
                    nc.tensor.matmul(
                        w_ps[:, n0:n1], ones[:], w_sb[:, n0:n1],
                        start=True, stop=True,
                    )
                if cw_gc:
                    nc.scalar.copy(w_bc[:, :cw_gc], w_ps[:, cw_d:cw])

            def emit_pass2_and_dma(st, ring):
                ob, c0, c1, cw, cw_d, cw_gc, w_ps, w_bc, w_sb = st
                if cw_gc:
                    nc.gpsimd.tensor_mul(
                        out=ob[:, :, cw_d:cw],
                        in0=ob[:, :, cw_d:cw],
                        in1=bcast_mid(w_bc[:, :cw_gc], bh),
                    )
                nc.vector.tensor_mul(
                    out=ob[:, :, :cw_d],
                    in0=ob[:, :, :cw_d],
                    in1=bcast_mid(w_ps[:, :cw_d], bh),
                )
                ring.dma_start(out=out_r[:, :, c0:c1], in_=ob[:, :, :cw])

            prev = None
            for k, (c0, c1) in enumerate(sched):
                cw = c1 - c0
                scale = cw / CHUNK
                cw_gc = int(cw * P2G_FRAC) if cw >= 512 else 0
                cw_d = cw - cw_gc

                w_ps = pp.tile([128, CHUNK], f32, tag="wps")
                w_sb = wp.tile([1, CHUNK], f16, tag="w")
                nc.scalar.dma_start(out=w_sb[:, :cw], in_=w_d[:, c0:c1])
                if cw_gc:
                    w_bc = wb.tile([128, int(CHUNK * P2G_FRAC) + 8], f32,
                                   tag="wbc")
                else:
                    w_bc = None
                ob = op.tile([128, bh, CHUNK], f32, tag="ob")
                st = (ob, c0, c1, cw, cw_d, cw_gc, w_ps, w_bc, w_sb)

                if k <= 1:
                    emit_w_broadcast(st)

                act_ns = gps_ns = 0.0
                act_cap = T_ACT_NS * scale
                gps_cap = T_GPS1_NS * scale
                for i, ps, pe, j0 in chunk_segments(f, c0, c1):
                    ln = pe - ps
                    o0 = ps - c0
                    na = 0
                    while na < bh and act_ns + ACT_UNIT_NS <= act_cap:
                        na += 1
                        act_ns += ACT_UNIT_NS
                    ng = 0
                    while na + ng < bh and gps_ns + ln * GPS_ELEM_NS <= gps_cap:
                        ng += 1
                        gps_ns += ln * GPS_ELEM_NS
                    nd = bh - na - ng
                    for b in range(na):
                        nc.scalar.activation(
                            ob[:, b, o0:o0 + ln],
                            x_sb[:, b, j0:j0 + ln],
                            mybir.ActivationFunctionType.Copy,
                            scale=x_sb[:, b, i:i + 1],
                        )
                    if nd:
                        nc.vector.tensor_mul(
                            out=ob[:, na:na + nd, o0:o0 + ln],
                            in0=x_sb[:, na:na + nd, j0:j0 + ln],
                            in1=bcast_last(x_sb[:, na:na + nd, i:i + 1], ln),
                        )
                    if ng:
                        b0 = na + nd
                        nc.gpsimd.tensor_mul(
                            out=ob[:, b0:bh, o0:o0 + ln],
                            in0=x_sb[:, b0:bh, j0:j0 + ln],
                            in1=bcast_last(x_sb[:, b0:bh, i:i + 1], ln),
                        )

                if k == 0:
                    # lead-in piece runs unskewed so the first out-DMA is
                    # not queued behind chunk 1's pass-1 on DVE/GPSIMD
                    emit_pass2_and_dma(st, nc.sync)
                else:
                    if prev is not None:
                        ring = nc.sync if k % 2 == 0 else nc.scalar
                        emit_pass2_and_dma(prev, ring)
                    if k > 1:
                        emit_w_broadcast(st)
                    prev = st
            ring = nc.sync if len(sched) % 2 == 0 else nc.scalar
            emit_pass2_and_dma(prev, ring)

    nc.compile()
    return nc


def pair_weights(v):
    """w[p] = dot(v[i(p)], v[j(p)]) in row-major triu order, fp16, padded
    and reshaped to [128, W_COLS]."""
    g = v.astype(np.float64) @ v.astype(np.float64).T
    ii, jj = np.triu_indices(v.shape[0], k=1)
    return np.ascontiguousarray(g[ii, jj][None, :].astype(np.float16))


_prog_cache = {}


def _get_program():
    key = (N_CORES, F_FULL, CHUNK, T_ACT_NS, T_GPS1_NS, P2G_FRAC)
    if key not in _prog_cache:
        _prog_cache[key] = build_program()
    return _prog_cache[key]


def run(x, v, trace=False, trace_kwargs=None):
    """Run on all 8 cores; returns (out [8192, P] f32, BassKernelResults)."""
    assert x.shape == (B_FULL, F_FULL), x.shape
    nc = _get_program()
    w = pair_weights(np.asarray(v))
    xs = np.ascontiguousarray(np.asarray(x, dtype=np.float32))
    b_loc = B_FULL // N_CORES
    in_maps = [
        {"x": np.ascontiguousarray(xs[c * b_loc:(c + 1) * b_loc]), "w": w}
        for c in range(N_CORES)
    ]
    res = run_bass_kernel_spmd(
        nc, in_maps, list(range(N_CORES)), trace=trace,
        **(trace_kwargs or {}),
    )
    out = np.concatenate([res.results[c]["out"] for c in range(N_CORES)], axis=0)
    return out, res


def kernel(x, v):
    out, _ = run(x, v)
    return out


# revision 14
# speedup vs baseline: 1.2833x; 1.2833x over previous
"""Trainium2 Bass kernel for CrossFeature: out[b, p(i,j)] = x[b,i]*x[b,j]*dot(v[i],v[j]).

Full shapes: x [8192, 300] f32, v [300, 4] f32 -> out [8192, 44850] f32
(P = 300*299/2 upper-triangular pairs, row-major order).

Strategy (data-parallel over 8 NeuronCores, batch-sharded, no cross-core
communication; per core 1024 rows as [128 part, 8 bh, 300]):
  - host: w[p] = (v @ v.T)[i(p), j(p)] as fp16, reshaped [128, 351] so the
    whole table lives in SBUF from one tiny DMA (351 = 44928/128; P padded).
  - output columns in 351-aligned chunks. Per chunk:
      * PE broadcasts w into PSUM fp32: ones[1,128]^T @ w2[row, :351] per
        351-col block (w2 rows hold consecutive w slices; rhs partition
        offset = row).
      * pass 1 (t = x_i * x_seg): per segment, the 8 row-blocks (b) are
        split b-granularly across ScalarE (activation Copy w/ scale=x_i,
        per (i,b)), DVE and GPSIMD (tensor_tensor with stride-0 x_i
        broadcast over a contiguous b-range) by per-chunk time budgets, so
        every chunk has the same engine profile and all engines run below
        the DMA-bound chunk wall.
      * pass 2 (t *= w): one DVE TT [128, 8, cw_d] vs PSUM w (stride-0 mid
        dim) + GPSIMD TT on the last cw_gc columns from an SBUF w copy
        (ScalarE).
      * out DMA alternates between the two HWDGE rings (sync / scalar).
  - chunk schedule: small lead-in/lead-out pieces (fast pipeline ramp and
    drain), middle chunks interleaved (0,N-1,1,N-2,...) so ACT-heavy early
    chunks pair with GPSIMD-heavy late ones; pass-2/DMA of chunk k-1 and
    the PE broadcast of chunk k are emitted after pass-1 of chunk k
    (software pipelining; also makes the single PSUM buffer safe).
"""

import numpy as np

import concourse.bacc as bacc
import concourse.bass as bass
import concourse.mybir as mybir
from concourse.tile import TileContext
from concourse.bass_utils import run_bass_kernel_spmd

N_CORES = 8
B_FULL = 8192
F_FULL = 300

CHUNK = 1536          # output columns per main chunk
EDGE_PIECES = 512     # small pieces for pipeline lead-in/lead-out

# per-chunk engine budgets, scaled by cw/CHUNK
T_ACT_NS = 16500      # ScalarE pass-1 budget (517 ns/instr, count-bound)
T_GPS1_NS = 8500      # GPSIMD pass-1 budget (2.17 ns/elem)
ACT_UNIT_NS = 517
GPS_ELEM_NS = 2.17
P2G_FRAC = 0.29       # pass-2 column share for GPSIMD (vs DVE)


def bcast_last(ap, n):
    """[..., 1] AP -> [..., n] with stride-0 last dim (free-dim broadcast)."""
    a = [list(d) for d in ap.ap]
    assert a[-1][1] == 1, a
    return bass.AP(ap.tensor, ap.offset, a[:-1] + [[0, n]])


def bcast_mid(ap, n):
    """[p, m] AP -> [p, n, m] with a stride-0 middle dim."""
    a = [list(d) for d in ap.ap]
    return bass.AP(ap.tensor, ap.offset, a[:-1] + [[0, n]] + a[-1:])


def chunk_segments(f, c0, c1):
    """Pair-segments of the triu(f, k=1) row-major layout intersected with
    column window [c0, c1). Yields (i, ps, pe, j0): output cols [ps, pe) hold
    x[:, i] * x[:, j0 : j0 + (pe-ps)]."""
    s = 0
    for i in range(f - 1):
        ln = f - 1 - i
        s0, s1 = s, s + ln
        if s0 >= c1:
            break
        if s1 > c0:
            ps, pe = max(s0, c0), min(s1, c1)
            yield i, ps, pe, i + 1 + (ps - s0)
        s = s1


def chunk_schedule(p_pairs):
    """[(c0, c1), ...] in emission order: small lead-in pieces, interleaved
    middle chunks, small lead-out pieces. All boundaries 351-aligned."""
    bounds = []
    c = 0
    while c < p_pairs:
        first = c == 0
        step = EDGE_PIECES if first else CHUNK
        bounds.append((c, min(c + step, p_pairs)))
        c += step
    # split the last full chunk into small pieces too
    last = bounds.pop()
    c = last[0]
    while c < last[1]:
        bounds.append((c, min(c + EDGE_PIECES, last[1])))
        c += EDGE_PIECES
    n_lead = 1
    n_tail = len([b for b in bounds if b[0] >= last[0]])
    mid = bounds[n_lead:len(bounds) - n_tail]
    order = bounds[:n_lead]
    lo, hi = 0, len(mid) - 1
    while lo <= hi:
        order.append(mid[lo])
        if hi != lo:
            order.append(mid[hi])
        lo, hi = lo + 1, hi - 1
    order += bounds[len(bounds) - n_tail:]
    return order


def build_program(bh=8, f=F_FULL, n_cores=N_CORES):
    """Build + compile the per-core Bass program. Shard shape: [bh*128, f]."""
    p_pairs = f * (f - 1) // 2
    rows = bh * 128
    f32 = mybir.dt.float32
    f16 = mybir.dt.float16

    nc = bacc.Bacc("TRN2", target_bir_lowering=False, debug=False,
                   num_devices=n_cores)
    x_d = nc.dram_tensor("x", [rows, f], f32, kind="ExternalInput")
    w_d = nc.dram_tensor("w", [1, p_pairs], f16, kind="ExternalInput")
    o_d = nc.dram_tensor("out", [rows, p_pairs], f32, kind="ExternalOutput")

    sched = chunk_schedule(p_pairs)

    with TileContext(nc) as tc:
        with (
            tc.tile_pool(name="xp", bufs=1) as xp,
            tc.tile_pool(name="wp", bufs=3) as wp,
            tc.tile_pool(name="wb", bufs=2) as wb,
            tc.tile_pool(name="op", bufs=3) as op,
            tc.tile_pool(name="pp", bufs=2, space=bass.MemorySpace.PSUM) as pp,
        ):
            x_sb = xp.tile([128, bh, f], f32)
            nc.scalar.dma_start(
                out=x_sb[:],
                in_=x_d.rearrange("(bh bl) f -> bl bh f", bl=128),
            )
            ones = xp.tile([1, 128], f16)
            nc.vector.memset(ones[:], 1.0)

            out_r = o_d.rearrange("(bh bl) p -> bl bh p", bl=128)

            def emit_w_broadcast(st):
                _, c0, c1, cw, cw_d, cw_gc, w_ps, w_bc, w_sb = st
                for n0 in range(0, cw, 512):
                    n1 = min(n0 + 512, cw)
                    nc.tensor.matmul(
                        w_ps[:, n0:n1], ones[:], w_sb[:, n0:n1],
                        start=True, stop=True,
                    )
                if cw_gc:
                    nc.scalar.copy(w_bc[:, :cw_gc], w_ps[:, cw_d:cw])

            def emit_pass2_and_dma(st, ring):
                ob, c0, c1, cw, cw_d, cw_gc, w_ps, w_bc, w_sb = st
                if cw_gc:
                    nc.gpsimd.tensor_mul(
                        out=ob[:, :, cw_d:cw],
                        in0=ob[:, :, cw_d:cw],
                        in1=bcast_mid(w_bc[:, :cw_gc], bh),
                    )
                nc.vector.tensor_mul(
                    out=ob[:, :, :cw_d],
                    in0=ob[:, :, :cw_d],
                    in1=bcast_mid(w_ps[:, :cw_d], bh),
                )
                ring.dma_start(out=out_r[:, :, c0:c1], in_=ob[:, :, :cw])

            prev = None
            for k, (c0, c1) in enumerate(sched):
                cw = c1 - c0
                scale = cw / CHUNK
                cw_gc = int(cw * P2G_FRAC) if cw >= 512 else 0
                cw_d = cw - cw_gc

                w_ps = pp.tile([128, CHUNK], f32, tag="wps")
                w_sb = wp.tile([1, CHUNK], f16, tag="w")
                nc.scalar.dma_start(out=w_sb[:, :cw], in_=w_d[:, c0:c1])
                if cw_gc:
                    w_bc = wb.tile([128, int(CHUNK * P2G_FRAC) + 8], f32,
                                   tag="wbc")
                else:
                    w_bc = None
                ob = op.tile([128, bh, CHUNK], f32, tag="ob")
                st = (ob, c0, c1, cw, cw_d, cw_gc, w_ps, w_bc, w_sb)
                emit_w_broadcast(st)

                act_ns = gps_ns = 0.0
                act_cap = T_ACT_NS * scale
                gps_cap = T_GPS1_NS * scale
                for i, ps, pe, j0 in chunk_segments(f, c0, c1):
                    ln = pe - ps
                    o0 = ps - c0
                    na = 0
                    while na < bh and act_ns + ACT_UNIT_NS <= act_cap:
                        na += 1
                        act_ns += ACT_UNIT_NS
                    ng = 0
                    while na + ng < bh and gps_ns + ln * GPS_ELEM_NS <= gps_cap:
                        ng += 1
                        gps_ns += ln * GPS_ELEM_NS
                    nd = bh - na - ng
                    for b in range(na):
                        nc.scalar.activation(
                            ob[:, b, o0:o0 + ln],
                            x_sb[:, b, j0:j0 + ln],
                            mybir.ActivationFunctionType.Copy,
                            scale=x_sb[:, b, i:i + 1],
                        )
                    if nd:
                        nc.vector.tensor_mul(
                            out=ob[:, na:na + nd, o0:o0 + ln],
                            in0=x_sb[:, na:na + nd, j0:j0 + ln],
                            in1=bcast_last(x_sb[:, na:na + nd, i:i + 1], ln),
                        )
                    if ng:
                        b0 = na + nd
                        nc.gpsimd.tensor_mul(
                            out=ob[:, b0:bh, o0:o0 + ln],
                            in0=x_sb[:, b0:bh, j0:j0 + ln],
                            in1=bcast_last(x_sb[:, b0:bh, i:i + 1], ln),
                        )

                if k == 0:
                    # lead-in piece runs unskewed so the first out-DMA is
                    # not queued behind chunk 1's pass-1 on DVE/GPSIMD
                    emit_pass2_and_dma(st, nc.sync)
                else:
                    if prev is not None:
                        ring = nc.sync if k % 2 == 0 else nc.scalar
                        emit_pass2_and_dma(prev, ring)
                    prev = st
            ring = nc.sync if len(sched) % 2 == 0 else nc.scalar
            emit_pass2_and_dma(prev, ring)

    nc.compile()
    return nc


def pair_weights(v):
    """w[p] = dot(v[i(p)], v[j(p)]) in row-major triu order, fp16, padded
    and reshaped to [128, W_COLS]."""
    g = v.astype(np.float64) @ v.astype(np.float64).T
    ii, jj = np.triu_indices(v.shape[0], k=1)
    return np.ascontiguousarray(g[ii, jj][None, :].astype(np.float16))


_prog_cache = {}


def _get_program():
    key = (N_CORES, F_FULL, CHUNK, T_ACT_NS, T_GPS1_NS, P2G_FRAC)
    if key not in _prog_cache:
        _prog_cache[key] = build_program()
    return _prog_cache[key]


def run(x, v, trace=False, trace_kwargs=None):
    """Run on all 8 cores; returns (out [8192, P] f32, BassKernelResults)."""
    assert x.shape == (B_FULL, F_FULL), x.shape
    nc = _get_program()
    w = pair_weights(np.asarray(v))
    xs = np.ascontiguousarray(np.asarray(x, dtype=np.float32))
    b_loc = B_FULL // N_CORES
    in_maps = [
        {"x": np.ascontiguousarray(xs[c * b_loc:(c + 1) * b_loc]), "w": w}
        for c in range(N_CORES)
    ]
    res = run_bass_kernel_spmd(
        nc, in_maps, list(range(N_CORES)), trace=trace,
        **(trace_kwargs or {}),
    )
    out = np.concatenate([res.results[c]["out"] for c in range(N_CORES)], axis=0)
    return out, res


def kernel(x, v):
    out, _ = run(x, v)
    return out


# revision 15
# speedup vs baseline: 1.4201x; 1.1065x over previous
"""Trainium2 Bass kernel for CrossFeature: out[b, p(i,j)] = x[b,i]*x[b,j]*dot(v[i],v[j]).

Full shapes: x [8192, 300] f32, v [300, 4] f32 -> out [8192, 44850] f32
(P = 300*299/2 upper-triangular pairs, row-major order).

Strategy (data-parallel over 8 NeuronCores, batch-sharded, no cross-core
communication; per core 1024 rows as [128 part, 8 bh, 300]):
  - host: w[p] = (v @ v.T)[i(p), j(p)] as fp16, reshaped [128, 351] so the
    whole table lives in SBUF from one tiny DMA (351 = 44928/128; P padded).
  - output columns in 351-aligned chunks. Per chunk:
      * PE broadcasts w into PSUM fp32: ones[1,128]^T @ w2[row, :351] per
        351-col block (w2 rows hold consecutive w slices; rhs partition
        offset = row).
      * pass 1 (t = x_i * x_seg): per segment, the 8 row-blocks (b) are
        split b-granularly across ScalarE (activation Copy w/ scale=x_i,
        per (i,b)), DVE and GPSIMD (tensor_tensor with stride-0 x_i
        broadcast over a contiguous b-range) by per-chunk time budgets, so
        every chunk has the same engine profile and all engines run below
        the DMA-bound chunk wall.
      * pass 2 (t *= w): one DVE TT [128, 8, cw_d] vs PSUM w (stride-0 mid
        dim) + GPSIMD TT on the last cw_gc columns from an SBUF w copy
        (ScalarE).
      * out DMA alternates between the two HWDGE rings (sync / scalar).
  - chunk schedule: small lead-in/lead-out pieces (fast pipeline ramp and
    drain), middle chunks interleaved (0,N-1,1,N-2,...) so ACT-heavy early
    chunks pair with GPSIMD-heavy late ones; pass-2/DMA of chunk k-1 and
    the PE broadcast of chunk k are emitted after pass-1 of chunk k
    (software pipelining; also makes the single PSUM buffer safe).
"""

import numpy as np

import concourse.bacc as bacc
import concourse.bass as bass
import concourse.mybir as mybir
from concourse.tile import TileContext
from concourse.bass_utils import run_bass_kernel_spmd

N_CORES = 8
B_FULL = 8192
F_FULL = 300

CHUNK = 1536          # output columns per main chunk
EDGE_PIECES = 512     # small pieces for pipeline lead-in/lead-out

# per-chunk engine budgets, scaled by cw/CHUNK
T_ACT_NS = 16500      # ScalarE pass-1 budget (517 ns/instr, count-bound)
T_GPS1_NS = 8500      # GPSIMD pass-1 budget (2.17 ns/elem)
ACT_UNIT_NS = 517
GPS_ELEM_NS = 2.17
P2G_FRAC = 0.29       # pass-2 column share for GPSIMD (vs DVE)


def bcast_last(ap, n):
    """[..., 1] AP -> [..., n] with stride-0 last dim (free-dim broadcast)."""
    a = [list(d) for d in ap.ap]
    assert a[-1][1] == 1, a
    return bass.AP(ap.tensor, ap.offset, a[:-1] + [[0, n]])


def bcast_mid(ap, n):
    """[p, m] AP -> [p, n, m] with a stride-0 middle dim."""
    a = [list(d) for d in ap.ap]
    return bass.AP(ap.tensor, ap.offset, a[:-1] + [[0, n]] + a[-1:])


def chunk_segments(f, c0, c1):
    """Pair-segments of the triu(f, k=1) row-major layout intersected with
    column window [c0, c1). Yields (i, ps, pe, j0): output cols [ps, pe) hold
    x[:, i] * x[:, j0 : j0 + (pe-ps)]."""
    s = 0
    for i in range(f - 1):
        ln = f - 1 - i
        s0, s1 = s, s + ln
        if s0 >= c1:
            break
        if s1 > c0:
            ps, pe = max(s0, c0), min(s1, c1)
            yield i, ps, pe, i + 1 + (ps - s0)
        s = s1


def chunk_schedule(p_pairs):
    """[(c0, c1), ...] in emission order: small lead-in pieces, interleaved
    middle chunks, small lead-out pieces. All boundaries 351-aligned."""
    bounds = []
    c = 0
    while c < p_pairs:
        first = c == 0
        step = EDGE_PIECES if first else CHUNK
        bounds.append((c, min(c + step, p_pairs)))
        c += step
    # split the last full chunk into small pieces too
    last = bounds.pop()
    c = last[0]
    while c < last[1]:
        bounds.append((c, min(c + EDGE_PIECES, last[1])))
        c += EDGE_PIECES
    n_lead = 1
    n_tail = len([b for b in bounds if b[0] >= last[0]])
    mid = bounds[n_lead:len(bounds) - n_tail]
    order = bounds[:n_lead]
    lo, hi = 0, len(mid) - 1
    while lo <= hi:
        order.append(mid[lo])
        if hi != lo:
            order.append(mid[hi])
        lo, hi = lo + 1, hi - 1
    order += bounds[len(bounds) - n_tail:]
    return order


def build_program(bh=8, f=F_FULL, n_cores=N_CORES):
    """Build + compile the per-core Bass program. Shard shape: [bh*128, f]."""
    p_pairs = f * (f - 1) // 2
    rows = bh * 128
    f32 = mybir.dt.float32
    f16 = mybir.dt.float16

    nc = bacc.Bacc("TRN2", target_bir_lowering=False, debug=False,
                   num_devices=n_cores)
    x_d = nc.dram_tensor("x", [rows, f], f32, kind="ExternalInput")
    w_d = nc.dram_tensor("w", [1, p_pairs], f16, kind="ExternalInput")
    o_d = nc.dram_tensor("out", [rows, p_pairs], f32, kind="ExternalOutput")

    sched = chunk_schedule(p_pairs)

    with TileContext(nc) as tc:
        with (
            tc.tile_pool(name="xp", bufs=1) as xp,
            tc.tile_pool(name="wp", bufs=3) as wp,
            tc.tile_pool(name="wb", bufs=2) as wb,
            tc.tile_pool(name="op", bufs=3) as op,
            tc.tile_pool(name="pp", bufs=2, space=bass.MemorySpace.PSUM) as pp,
        ):
            x_sb = xp.tile([128, bh, f], f32)
            # row (bl*8 + b) <-> x_sb[bl, b, :]: each partition loads 8
            # consecutive DRAM rows = one contiguous 9.6KB descriptor
            nc.scalar.dma_start(
                out=x_sb[:],
                in_=x_d.rearrange("(bl b) f -> bl b f", bl=128),
            )
            ones = xp.tile([1, 128], f16)
            nc.vector.memset(ones[:], 1.0)

            out_r = o_d.rearrange("(bl b) p -> bl b p", bl=128)

            def emit_w_broadcast(st):
                _, c0, c1, cw, cw_d, cw_gc, w_ps, w_bc, w_sb = st
                for n0 in range(0, cw, 512):
                    n1 = min(n0 + 512, cw)
                    nc.tensor.matmul(
                        w_ps[:, n0:n1], ones[:], w_sb[:, n0:n1],
                        start=True, stop=True,
                    )
                if cw_gc:
                    nc.scalar.copy(w_bc[:, :cw_gc], w_ps[:, cw_d:cw])

            def emit_pass2_and_dma(st, ring):
                ob, c0, c1, cw, cw_d, cw_gc, w_ps, w_bc, w_sb = st
                if cw_gc:
                    nc.gpsimd.tensor_mul(
                        out=ob[:, :, cw_d:cw],
                        in0=ob[:, :, cw_d:cw],
                        in1=bcast_mid(w_bc[:, :cw_gc], bh),
                    )
                nc.vector.tensor_mul(
                    out=ob[:, :, :cw_d],
                    in0=ob[:, :, :cw_d],
                    in1=bcast_mid(w_ps[:, :cw_d], bh),
                )
                ring.dma_start(out=out_r[:, :, c0:c1], in_=ob[:, :, :cw])

            prev = None
            for k, (c0, c1) in enumerate(sched):
                cw = c1 - c0
                scale = cw / CHUNK
                cw_gc = int(cw * P2G_FRAC) if cw >= 512 else 0
                cw_d = cw - cw_gc

                w_ps = pp.tile([128, CHUNK], f32, tag="wps")
                w_sb = wp.tile([1, CHUNK], f16, tag="w")
                nc.scalar.dma_start(out=w_sb[:, :cw], in_=w_d[:, c0:c1])
                if cw_gc:
                    w_bc = wb.tile([128, int(CHUNK * P2G_FRAC) + 8], f32,
                                   tag="wbc")
                else:
                    w_bc = None
                ob = op.tile([128, bh, CHUNK], f32, tag="ob")
                st = (ob, c0, c1, cw, cw_d, cw_gc, w_ps, w_bc, w_sb)
                emit_w_broadcast(st)

                act_ns = gps_ns = 0.0
                act_cap = T_ACT_NS * scale
                gps_cap = T_GPS1_NS * scale
                for i, ps, pe, j0 in chunk_segments(f, c0, c1):
                    ln = pe - ps
                    o0 = ps - c0
                    na = 0
                    while na < bh and act_ns + ACT_UNIT_NS <= act_cap:
                        na += 1
                        act_ns += ACT_UNIT_NS
                    ng = 0
                    while na + ng < bh and gps_ns + ln * GPS_ELEM_NS <= gps_cap:
                        ng += 1
                        gps_ns += ln * GPS_ELEM_NS
                    nd = bh - na - ng
                    for b in range(na):
                        nc.scalar.activation(
                            ob[:, b, o0:o0 + ln],
                            x_sb[:, b, j0:j0 + ln],
                            mybir.ActivationFunctionType.Copy,
                            scale=x_sb[:, b, i:i + 1],
                        )
                    if nd:
                        nc.vector.tensor_mul(
                            out=ob[:, na:na + nd, o0:o0 + ln],
                            in0=x_sb[:, na:na + nd, j0:j0 + ln],
                            in1=bcast_last(x_sb[:, na:na + nd, i:i + 1], ln),
                        )
                    if ng:
                        b0 = na + nd
                        nc.gpsimd.tensor_mul(
                            out=ob[:, b0:bh, o0:o0 + ln],
                            in0=x_sb[:, b0:bh, j0:j0 + ln],
                            in1=bcast_last(x_sb[:, b0:bh, i:i + 1], ln),
                        )

                if k == 0:
                    # lead-in piece runs unskewed so the first out-DMA is
                    # not queued behind chunk 1's pass-1 on DVE/GPSIMD
                    emit_pass2_and_dma(st, nc.sync)
                else:
                    if prev is not None:
                        ring = nc.sync if k % 2 == 0 else nc.scalar
                        emit_pass2_and_dma(prev, ring)
                    prev = st
            ring = nc.sync if len(sched) % 2 == 0 else nc.scalar
            emit_pass2_and_dma(prev, ring)

    nc.compile()
    return nc


def pair_weights(v):
    """w[p] = dot(v[i(p)], v[j(p)]) in row-major triu order, fp16, padded
    and reshaped to [128, W_COLS]."""
    g = v.astype(np.float64) @ v.astype(np.float64).T
    ii, jj = np.triu_indices(v.shape[0], k=1)
    return np.ascontiguousarray(g[ii, jj][None, :].astype(np.float16))


_prog_cache = {}


def _get_program():
    key = (N_CORES, F_FULL, CHUNK, T_ACT_NS, T_GPS1_NS, P2G_FRAC)
    if key not in _prog_cache:
        _prog_cache[key] = build_program()
    return _prog_cache[key]


def run(x, v, trace=False, trace_kwargs=None):
    """Run on all 8 cores; returns (out [8192, P] f32, BassKernelResults)."""
    assert x.shape == (B_FULL, F_FULL), x.shape
    nc = _get_program()
    w = pair_weights(np.asarray(v))
    xs = np.ascontiguousarray(np.asarray(x, dtype=np.float32))
    b_loc = B_FULL // N_CORES
    in_maps = [
        {"x": np.ascontiguousarray(xs[c * b_loc:(c + 1) * b_loc]), "w": w}
        for c in range(N_CORES)
    ]
    res = run_bass_kernel_spmd(
        nc, in_maps, list(range(N_CORES)), trace=trace,
        **(trace_kwargs or {}),
    )
    out = np.concatenate([res.results[c]["out"] for c in range(N_CORES)], axis=0)
    return out, res


def kernel(x, v):
    out, _ = run(x, v)
    return out
